# revision 6
# baseline (speedup 1.0000x reference)
# GraphSAGE 2-layer GNN on 8 TRN2 NeuronCores.
#
# Strategy (graph/data parallel, per sharding hint):
#   - dst-partition nodes across 8 cores (6250 rows each).
#   - Host: sort edges by (core, window, src), fold 1/(deg+eps) into per-edge
#     weights, build int16 gather-index streams + per-chunk one-hot metadata,
#     padded to a uniform max structure so all cores run one SPMD program.
#   - Device, per layer: bulk dma_gather of edge features (x rows fp32 /
#     h rows bf16) -> DVE builds weighted one-hot [128 edges x 256 rows] ->
#     TensorE segment-sum matmul into PSUM -> weight matmul -> bias(+relu)
#     on ACT -> PE transpose -> SBUF-resident h -> AllGather (bf16) ->
#     layer 2 -> batched log_softmax epilogue.
import sys

sys.path.insert(0, "/opt/trn_rl_repo")

import numpy as np
import ml_dtypes

import concourse.bass as bass
import concourse.bacc as bacc
import concourse.mybir as mybir
import concourse.tile as tile
from concourse.bass_utils import run_bass_kernel_spmd

F32 = mybir.dt.float32
F32R = mybir.dt.float32r
BF16 = mybir.dt.bfloat16
I16 = mybir.dt.int16


class Cfg:
    def __init__(self, N=50000, E=800000, F1=64, F2=128, F3=64, ncores=8,
                 win=256, lo_lim=32768, piece_chunks=128):
        self.N, self.E = N, E
        self.F1, self.F2, self.F3 = F1, F2, F3
        self.NC = ncores
        self.WIN = win
        self.LO = lo_lim
        self.PIECE = piece_chunks
        self.RPC = N // ncores               # rows per core
        assert self.RPC * ncores == N
        self.NWIN = -(-self.RPC // win)      # windows per core
        self.HALVES = -(-self.RPC // 128)    # 128-row halves per core
        self.RPAD = self.HALVES * 128        # padded rows per core
        self.NPAD = self.RPAD * ncores


def prep(cfg, src, dst, deg_w):
    """Build per-core gather/one-hot metadata with a core-uniform structure.

    Returns (struct, per_core) where struct has the shared max-shape info the
    program builder needs, and per_core the numpy arrays for in_maps.
    """
    NC, WIN, LO, RPC, RPAD = cfg.NC, cfg.WIN, cfg.LO, cfg.RPC, cfg.RPAD
    pid = ((src // RPC) * RPAD + (src % RPC)).astype(np.int64)
    core = dst // RPC

    per_core_ed = []
    nlo = np.zeros((NC, cfg.NWIN), np.int64)
    nhi = np.zeros((NC, cfg.NWIN), np.int64)
    for c in range(NC):
        m = core == c
        sp = pid[m]
        dl = (dst[m] - c * RPC).astype(np.int64)
        wv = deg_w[dst[m]]
        wi = dl // WIN
        row = dl % WIN
        hi = (sp >= LO).astype(np.int64)
        order = np.lexsort((sp, hi, wi))
        sp, wv, wi, row, hi = sp[order], wv[order], wi[order], row[order], hi[order]
        per_core_ed.append((sp, wv, wi, row, hi))
        for w in range(cfg.NWIN):
            sel = wi == w
            nlo[c, w] = int((hi[sel] == 0).sum())
            nhi[c, w] = int(hi[sel].sum())

    # uniform chunk structure: per window, max #chunks across cores
    ch_lo = (-(-nlo.max(axis=0) // 128)).astype(np.int64)
    ch_hi = (-(-nhi.max(axis=0) // 128)).astype(np.int64)
    ch_lo = np.maximum(ch_lo, 1)
    ch_hi = np.maximum(ch_hi, 1)
    tot_lo, tot_hi = int(ch_lo.sum()), int(ch_hi.sum())
    totch = tot_lo + tot_hi

    # pieces: consecutive windows with total chunks <= PIECE
    pieces = []
    w0 = 0
    acc = 0
    for w in range(cfg.NWIN):
        cw = int(ch_lo[w] + ch_hi[w])
        assert cw <= cfg.PIECE, "single window exceeds piece budget"
        if acc + cw > cfg.PIECE:
            pieces.append((w0, w))
            w0, acc = w, 0
        acc += cw
    pieces.append((w0, cfg.NWIN))

    LB = np.concatenate([[0], np.cumsum(ch_lo)])   # lo-chunk base per window
    HB = np.concatenate([[0], np.cumsum(ch_hi)])
    MB = np.concatenate([[0], np.cumsum(ch_lo + ch_hi)])  # meta col base

    per_core = []
    for c in range(NC):
        sp, wv, wi, row, hi = per_core_ed[c]
        idx_lo = np.zeros(tot_lo * 128, np.int16)
        idx_hi = np.zeros(tot_hi * 128, np.int16)
        dstrow = np.full((totch * 128,), WIN, np.float32)  # sentinel row
        wgt = np.zeros((totch * 128,), np.float32)
        for w in range(cfg.NWIN):
            sel = wi == w
            sl = sel & (hi == 0)
            sh = sel & (hi == 1)
            klo, khi = int(sl.sum()), int(sh.sum())
            # lo stream
            b = LB[w] * 128
            idx_lo[b:b + klo] = sp[sl].astype(np.int16)
            # hi stream
            b = HB[w] * 128
            idx_hi[b:b + khi] = (sp[sh] - LO).astype(np.int16)
            # meta: lo chunks then hi chunks of this window
            b = MB[w] * 128
            dstrow[b:b + klo] = row[sl].astype(np.float32)
            wgt[b:b + klo] = wv[sl]
            b = (MB[w] + ch_lo[w]) * 128
            dstrow[b:b + khi] = row[sh].astype(np.float32)
            wgt[b:b + khi] = wv[sh]
        # idx arrays -> [16, n/16] interleave (idx i at [i%16, i//16])
        idx = np.concatenate([idx_lo, idx_hi])
        idx16 = np.tile(idx.reshape(-1, 16).T, (8, 1)).copy()
        per_core.append(dict(
            idx=idx16,
            dstrow=dstrow.reshape(-1, 128).T.copy(),
            wgt=wgt.reshape(-1, 128).T.copy(),
        ))

    struct = dict(ch_lo=ch_lo, ch_hi=ch_hi, tot_lo=tot_lo, tot_hi=tot_hi,
                  totch=totch, pieces=pieces, LB=LB, HB=HB, MB=MB)
    return struct, per_core


def build_program(cfg, struct):
    NC = cfg.NC
    F1, F2, F3, WIN = cfg.F1, cfg.F2, cfg.F3, cfg.WIN
    ch_lo, ch_hi = struct["ch_lo"], struct["ch_hi"]
    LB, HB, MB = struct["LB"], struct["HB"], struct["MB"]
    pieces = struct["pieces"]
    totch = struct["totch"]
    tot_lo = struct["tot_lo"]
    HALVES = cfg.HALVES

    nc = bacc.Bacc("TRN2", target_bir_lowering=False, debug=False,
                   num_devices=NC)

    x_lo = nc.dram_tensor("x_lo", [cfg.LO, F1], F32R, kind="ExternalInput")
    x_hi = nc.dram_tensor("x_hi", [cfg.NPAD - cfg.LO, F1], F32R,
                          kind="ExternalInput")
    W1b = nc.dram_tensor("W1b", [F1, F2], BF16, kind="ExternalInput")
    W2b = nc.dram_tensor("W2b", [F2, F3], BF16, kind="ExternalInput")
    b1d = nc.dram_tensor("b1d", [F2, 1], F32, kind="ExternalInput")
    b2d = nc.dram_tensor("b2d", [F3, 1], F32, kind="ExternalInput")
    iota32 = nc.dram_tensor("iota32", [128, WIN], F32, kind="ExternalInput")
    iota16 = nc.dram_tensor("iota16", [128, WIN], BF16, kind="ExternalInput")
    ident16 = nc.dram_tensor("ident16", [128, 128], BF16, kind="ExternalInput")
    ident32 = nc.dram_tensor("ident32", [128, 128], F32, kind="ExternalInput")
    idx_d = nc.dram_tensor("idx", [128, totch * 8], I16, kind="ExternalInput")
    dstrow_d = nc.dram_tensor("dstrow", [128, totch], F32, kind="ExternalInput")
    wgt_d = nc.dram_tensor("wgt", [128, totch], F32, kind="ExternalInput")
    out_d = nc.dram_tensor("out", [128, HALVES, F3], F32, kind="ExternalOutput")

    with tile.TileContext(nc) as tc:
        with (
            tc.tile_pool(name="const", bufs=1) as cpool,
            tc.tile_pool(name="persist", bufs=1) as ppool,
            tc.tile_pool(name="dram", bufs=1, space="DRAM") as dpool,
        ):
            io32 = cpool.tile([128, WIN], F32)
            nc.sync.dma_start(out=io32[:], in_=iota32[:])
            io16 = cpool.tile([128, WIN], BF16)
            nc.sync.dma_start(out=io16[:], in_=iota16[:])
            w1 = cpool.tile([F1, F2], BF16)
            nc.sync.dma_start(out=w1[:], in_=W1b[:])
            w2 = cpool.tile([F2, F3], BF16)
            nc.sync.dma_start(out=w2[:], in_=W2b[:])
            b1 = cpool.tile([F2, 1], F32)
            nc.sync.dma_start(out=b1[:], in_=b1d[:])
            b2 = cpool.tile([F3, 1], F32)
            nc.sync.dma_start(out=b2[:], in_=b2d[:])
            id16 = cpool.tile([128, 128], BF16)
            nc.sync.dma_start(out=id16[:], in_=ident16[:])
            id32 = cpool.tile([128, 128], F32)
            nc.sync.dma_start(out=id32[:], in_=ident32[:])
            idxs = cpool.tile([128, totch * 8], I16)
            nc.sync.dma_start(out=idxs[:], in_=idx_d[:])
            dstrow = cpool.tile([128, totch], F32)
            nc.sync.dma_start(out=dstrow[:], in_=dstrow_d[:])
            wgt = cpool.tile([128, totch], F32)
            nc.sync.dma_start(out=wgt[:], in_=wgt_d[:])

            h_sb = ppool.tile([128, HALVES, F2], BF16)     # layer-1 output rows
            out_sb = ppool.tile([128, HALVES, F3], F32)    # layer-2 logits

            h_shard = dpool.tile([cfg.RPAD, F2], BF16)
            h_full = dpool.tile([cfg.NPAD, F2], BF16)

            def gather(dst_tile, in_ap, col0, n_chunks, elem):
                nc.gpsimd.dma_gather(
                    out_ap=dst_tile[:, :, :],
                    in_ap=in_ap,
                    idxs_ap=idxs[:, col0 * 8:(col0 + n_chunks) * 8],
                    num_idxs=n_chunks * 128,
                    num_idxs_reg=n_chunks * 128,
                    elem_size=elem,
                    single_packet=False,
                )

            def run_layer(layer):
                if layer == 1:
                    elem, gdt, src_lo, src_hi = F1, F32R, x_lo[:, :], x_hi[:, :]
                else:
                    elem, gdt = F2, BF16
                    src_lo = h_full[0:cfg.LO, :]
                    src_hi = h_full[cfg.LO:cfg.NPAD, :]
                with (
                    tc.tile_pool(name=f"g{layer}", bufs=2) as gpool,
                    tc.tile_pool(name=f"oh{layer}", bufs=8) as ohpool,
                    tc.tile_pool(name=f"ep{layer}", bufs=2) as eppool,
                    tc.tile_pool(name=f"ps{layer}", bufs=2, space="PSUM") as pspool,
                    tc.tile_pool(name=f"pt{layer}", bufs=2, space="PSUM") as ptpool,
                ):
                    for (w0, w1_) in pieces:
                        ncl = int(LB[w1_] - LB[w0])
                        nch = int(HB[w1_] - HB[w0])
                        g_lo = gpool.tile([128, ncl, elem], gdt, tag="glo")
                        g_hi = gpool.tile([128, nch, elem], gdt, tag="ghi")
                        gather(g_lo, src_lo, int(LB[w0]), ncl, elem)
                        gather(g_hi, src_hi, int(tot_lo + HB[w0]), nch, elem)
                        for w in range(w0, w1_):
                            nl, nh = int(ch_lo[w]), int(ch_hi[w])
                            if layer == 1:
                                acc = pspool.tile([F1, WIN], F32, tag="acc")
                            else:
                                acc = pspool.tile([F2, WIN], F32, tag="acc")
                            tot = nl + nh
                            for k in range(tot):
                                if k < nl:
                                    g = g_lo[:, int(LB[w] - LB[w0]) + k, :]
                                else:
                                    g = g_hi[:, int(HB[w] - HB[w0]) + k - nl, :]
                                mc = int(MB[w]) + k
                                if layer == 1:
                                    oh = ohpool.tile([128, WIN], F32R, tag="oh")
                                    nc.vector.tensor_scalar(
                                        out=oh[:], in0=io32[:],
                                        scalar1=dstrow[:, mc:mc + 1],
                                        scalar2=wgt[:, mc:mc + 1],
                                        op0=mybir.AluOpType.is_equal,
                                        op1=mybir.AluOpType.mult)
                                    nc.tensor.matmul(
                                        out=acc[:], lhsT=g, rhs=oh[:],
                                        start=(k == 0), stop=(k == tot - 1))
                                else:
                                    oh = ohpool.tile([128, WIN], BF16, tag="oh")
                                    nc.vector.tensor_scalar(
                                        out=oh[:], in0=io16[:],
                                        scalar1=dstrow[:, mc:mc + 1],
                                        scalar2=wgt[:, mc:mc + 1],
                                        op0=mybir.AluOpType.is_equal,
                                        op1=mybir.AluOpType.mult)
                                    nc.tensor.matmul(
                                        out=acc[:], lhsT=g, rhs=oh[:],
                                        start=(k == 0), stop=(k == tot - 1))
                            # window epilogue
                            import os as _os
                            if layer == 2 and _os.environ.get("K_STAGE") == "l2seg":
                                continue
                            if layer == 1:
                                mbf = eppool.tile([F1, WIN], BF16, tag="mbf")
                                nc.vector.tensor_copy(out=mbf[:], in_=acc[:])
                                z = ptpool.tile([F2, WIN], F32, tag="z")
                                nc.tensor.matmul(out=z[:], lhsT=w1[:],
                                                 rhs=mbf[:], start=True,
                                                 stop=True)
                                ht = eppool.tile([F2, WIN], BF16, tag="ht")
                                nc.scalar.activation(
                                    out=ht[:], in_=z[:],
                                    func=mybir.ActivationFunctionType.Relu,
                                    bias=b1[:, 0:1])
                                for hf in range(WIN // 128):
                                    hh = w * (WIN // 128) + hf
                                    if hh >= HALVES:
                                        continue
                                    tp = ptpool.tile([128, 128], BF16, tag="tp")
                                    nc.tensor.transpose(
                                        out=tp[:],
                                        in_=ht[:, hf * 128:(hf + 1) * 128],
                                        identity=id16[:])
                                    nc.vector.tensor_copy(
                                        out=h_sb[:, hh, :], in_=tp[:])
                            else:
                                mbf = eppool.tile([F2, WIN], BF16, tag="mbf")
                                nc.vector.tensor_copy(out=mbf[:], in_=acc[:])
                                z = ptpool.tile([F3, WIN], F32, tag="z")
                                nc.tensor.matmul(out=z[:], lhsT=w2[:],
                                                 rhs=mbf[:], start=True,
                                                 stop=True)
                                o2 = eppool.tile([128, WIN], F32, tag="ht")
                                nc.scalar.activation(
                                    out=o2[0:F3, :], in_=z[:],
                                    func=mybir.ActivationFunctionType.Identity,
                                    bias=b2[:, 0:1])
                                for hf in range(WIN // 128):
                                    hh = w * (WIN // 128) + hf
                                    if hh >= HALVES:
                                        continue
                                    tp = ptpool.tile([128, 128], F32, tag="tp")
                                    nc.tensor.transpose(
                                        out=tp[:],
                                        in_=o2[:, hf * 128:(hf + 1) * 128],
                                        identity=id32[:])
                                    nc.vector.tensor_copy(
                                        out=out_sb[:, hh, :], in_=tp[:, 0:F3])

            import os
            stage = os.environ.get("K_STAGE", "full")
            run_layer(1)
            if stage != "l1":
                # ship h to DRAM (strided: row hh*128+p), allgather, layer 2
                nc.sync.dma_start(
                    out=h_shard[:].rearrange("(hh p) f -> p hh f", p=128),
                    in_=h_sb[:])
                nc.gpsimd.collective_compute(
                    "AllGather",
                    mybir.AluOpType.bypass,
                    replica_groups=[list(range(NC))],
                    ins=[h_shard[:].opt()],
                    outs=[h_full[:].opt()],
                )
            if stage == "full":
                run_layer(2)

            # batched log_softmax over all halves: out = t - ln(sum(exp(t)))
            with tc.tile_pool(name="sm", bufs=1) as smp:
                mx = smp.tile([128, HALVES, 1], F32)
                nc.vector.reduce_max(out=mx[:], in_=out_sb[:],
                                     axis=mybir.AxisListType.X)
                nc.vector.tensor_tensor(
                    out=out_sb[:], in0=out_sb[:],
                    in1=mx[:].to_broadcast([128, HALVES, F3]),
                    op=mybir.AluOpType.subtract)
                ex = smp.tile([128, HALVES, F3], F32)
                nc.scalar.activation(out=ex[:], in_=out_sb[:],
                                     func=mybir.ActivationFunctionType.Exp)
                sm = smp.tile([128, HALVES, 1], F32)
                nc.vector.reduce_sum(out=sm[:], in_=ex[:],
                                     axis=mybir.AxisListType.X)
                ls = smp.tile([128, HALVES, 1], F32)
                nc.scalar.activation(out=ls[:], in_=sm[:],
                                     func=mybir.ActivationFunctionType.Ln)
                nc.vector.tensor_tensor(
                    out=out_sb[:], in0=out_sb[:],
                    in1=ls[:].to_broadcast([128, HALVES, F3]),
                    op=mybir.AluOpType.subtract)
                nc.sync.dma_start(out=out_d[:], in_=out_sb[:])

    nc.compile()
    return nc


_CACHE = {}


def _get_program(cfg, x, src, dst, W1, b1, W2, b2):
    deg = np.bincount(dst, minlength=cfg.N).astype(np.float64)
    deg_w = (1.0 / (deg + 1e-6)).astype(np.float32)
    struct, per_core = prep(cfg, src, dst, deg_w)

    xp = np.zeros((cfg.NPAD, cfg.F1), np.float32)
    for c in range(cfg.NC):
        xp[c * cfg.RPAD:c * cfg.RPAD + cfg.RPC] = x[c * cfg.RPC:(c + 1) * cfg.RPC]

    iota = np.arange(cfg.WIN, dtype=np.float32)
    shared = dict(
        x_lo=xp[:cfg.LO].copy(),
        x_hi=xp[cfg.LO:].copy(),
        W1b=W1.astype(ml_dtypes.bfloat16),
        W2b=W2.astype(ml_dtypes.bfloat16),
        b1d=b1.reshape(-1, 1).astype(np.float32),
        b2d=b2.reshape(-1, 1).astype(np.float32),
        iota32=np.tile(iota, (128, 1)),
        iota16=np.tile(iota, (128, 1)).astype(ml_dtypes.bfloat16),
        ident16=np.eye(128, dtype=ml_dtypes.bfloat16),
        ident32=np.eye(128, dtype=np.float32),
    )
    in_maps = []
    for c in range(cfg.NC):
        m = dict(shared)
        m["idx"] = per_core[c]["idx"]
        m["dstrow"] = per_core[c]["dstrow"]
        m["wgt"] = per_core[c]["wgt"]
        in_maps.append(m)

    key = (cfg.N, cfg.E, struct["totch"], tuple(struct["ch_lo"]),
           tuple(struct["ch_hi"]))
    if key not in _CACHE:
        _CACHE[key] = build_program(cfg, struct)
    return _CACHE[key], in_maps


def run(cfg, x, src, dst, W1, b1, W2, b2, trace=False, trace_kwargs=None):
    nc, in_maps = _get_program(cfg, x, src, dst, W1, b1, W2, b2)
    res = run_bass_kernel_spmd(nc, in_maps, core_ids=list(range(cfg.NC)),
                               trace=trace, **(trace_kwargs or {}))
    out = np.empty((cfg.N, cfg.F3), np.float32)
    for c in range(cfg.NC):
        o = np.asarray(res.results[c]["out"])  # [128, HALVES, F3]
        o = o.transpose(1, 0, 2).reshape(cfg.RPAD, cfg.F3)
        out[c * cfg.RPC:(c + 1) * cfg.RPC] = o[:cfg.RPC]
    return out, res


def kernel(x, src, dst, W1, b1, W2, b2):
    cfg = Cfg()
    out, _ = run(cfg, np.asarray(x, np.float32), np.asarray(src),
                 np.asarray(dst), np.asarray(W1, np.float32),
                 np.asarray(b1, np.float32), np.asarray(W2, np.float32),
                 np.asarray(b2, np.float32))
    return out



# revision 8
# speedup vs baseline: 1.0743x; 1.0743x over previous
# GraphSAGE 2-layer GNN on 8 TRN2 NeuronCores.
#
# Strategy (graph/data parallel, per sharding hint):
#   - dst-partition nodes across 8 cores (6250 rows each).
#   - Host: sort edges by (core, window, src), fold 1/(deg+eps) into per-edge
#     weights, build int16 gather-index streams + per-chunk one-hot metadata,
#     padded to a uniform max structure so all cores run one SPMD program.
#   - Device, per layer: bulk dma_gather of edge features (x rows fp32 /
#     h rows bf16) -> DVE builds weighted one-hot [128 edges x 256 rows] ->
#     TensorE segment-sum matmul into PSUM -> weight matmul -> bias(+relu)
#     on ACT -> PE transpose -> SBUF-resident h -> AllGather (bf16) ->
#     layer 2 -> batched log_softmax epilogue.
import sys

sys.path.insert(0, "/opt/trn_rl_repo")

import numpy as np
import ml_dtypes

import concourse.bass as bass
import concourse.bacc as bacc
import concourse.mybir as mybir
import concourse.tile as tile
from concourse.bass_utils import run_bass_kernel_spmd

F32 = mybir.dt.float32
F32R = mybir.dt.float32r
BF16 = mybir.dt.bfloat16
I16 = mybir.dt.int16


class Cfg:
    def __init__(self, N=50000, E=800000, F1=64, F2=128, F3=64, ncores=8,
                 win=256, lo_lim=32768, piece_chunks=64):
        self.N, self.E = N, E
        self.F1, self.F2, self.F3 = F1, F2, F3
        self.NC = ncores
        self.WIN = win
        self.LO = lo_lim
        self.PIECE = piece_chunks
        self.RPC = N // ncores               # rows per core
        assert self.RPC * ncores == N
        self.NWIN = -(-self.RPC // win)      # windows per core
        self.HALVES = -(-self.RPC // 128)    # 128-row halves per core
        self.RPAD = self.HALVES * 128        # padded rows per core
        self.NPAD = self.RPAD * ncores


def prep(cfg, src, dst, deg_w):
    """Build per-core gather/one-hot metadata with a core-uniform structure.

    Returns (struct, per_core) where struct has the shared max-shape info the
    program builder needs, and per_core the numpy arrays for in_maps.
    """
    NC, WIN, LO, RPC, RPAD = cfg.NC, cfg.WIN, cfg.LO, cfg.RPC, cfg.RPAD
    pid = ((src // RPC) * RPAD + (src % RPC)).astype(np.int64)
    core = dst // RPC

    per_core_ed = []
    nlo = np.zeros((NC, cfg.NWIN), np.int64)
    nhi = np.zeros((NC, cfg.NWIN), np.int64)
    for c in range(NC):
        m = core == c
        sp = pid[m]
        dl = (dst[m] - c * RPC).astype(np.int64)
        wv = deg_w[dst[m]]
        wi = dl // WIN
        row = dl % WIN
        hi = (sp >= LO).astype(np.int64)
        order = np.lexsort((sp, hi, wi))
        sp, wv, wi, row, hi = sp[order], wv[order], wi[order], row[order], hi[order]
        per_core_ed.append((sp, wv, wi, row, hi))
        for w in range(cfg.NWIN):
            sel = wi == w
            nlo[c, w] = int((hi[sel] == 0).sum())
            nhi[c, w] = int(hi[sel].sum())

    # uniform chunk structure: per window, max #chunks across cores
    ch_lo = (-(-nlo.max(axis=0) // 128)).astype(np.int64)
    ch_hi = (-(-nhi.max(axis=0) // 128)).astype(np.int64)
    ch_lo = np.maximum(ch_lo, 1)
    ch_hi = np.maximum(ch_hi, 1)
    tot_lo, tot_hi = int(ch_lo.sum()), int(ch_hi.sum())
    totch = tot_lo + tot_hi

    # pieces: consecutive windows with total chunks <= PIECE
    pieces = []
    w0 = 0
    acc = 0
    for w in range(cfg.NWIN):
        cw = int(ch_lo[w] + ch_hi[w])
        assert cw <= cfg.PIECE, "single window exceeds piece budget"
        if acc + cw > cfg.PIECE:
            pieces.append((w0, w))
            w0, acc = w, 0
        acc += cw
    pieces.append((w0, cfg.NWIN))

    LB = np.concatenate([[0], np.cumsum(ch_lo)])   # lo-chunk base per window
    HB = np.concatenate([[0], np.cumsum(ch_hi)])
    MB = np.concatenate([[0], np.cumsum(ch_lo + ch_hi)])  # meta col base

    per_core = []
    for c in range(NC):
        sp, wv, wi, row, hi = per_core_ed[c]
        idx_lo = np.zeros(tot_lo * 128, np.int16)
        idx_hi = np.zeros(tot_hi * 128, np.int16)
        dstrow = np.full((totch * 128,), WIN, np.float32)  # sentinel row
        wgt = np.zeros((totch * 128,), np.float32)
        for w in range(cfg.NWIN):
            sel = wi == w
            sl = sel & (hi == 0)
            sh = sel & (hi == 1)
            klo, khi = int(sl.sum()), int(sh.sum())
            # lo stream
            b = LB[w] * 128
            idx_lo[b:b + klo] = sp[sl].astype(np.int16)
            # hi stream
            b = HB[w] * 128
            idx_hi[b:b + khi] = (sp[sh] - LO).astype(np.int16)
            # meta: lo chunks then hi chunks of this window
            b = MB[w] * 128
            dstrow[b:b + klo] = row[sl].astype(np.float32)
            wgt[b:b + klo] = wv[sl]
            b = (MB[w] + ch_lo[w]) * 128
            dstrow[b:b + khi] = row[sh].astype(np.float32)
            wgt[b:b + khi] = wv[sh]
        # idx arrays -> [16, n/16] interleave (idx i at [i%16, i//16])
        idx = np.concatenate([idx_lo, idx_hi])
        idx16 = np.tile(idx.reshape(-1, 16).T, (8, 1)).copy()
        per_core.append(dict(
            idx=idx16,
            dstrow=dstrow.reshape(-1, 128).T.copy(),
            wgt=wgt.reshape(-1, 128).T.copy(),
        ))

    struct = dict(ch_lo=ch_lo, ch_hi=ch_hi, tot_lo=tot_lo, tot_hi=tot_hi,
                  totch=totch, pieces=pieces, LB=LB, HB=HB, MB=MB)
    return struct, per_core


def build_program(cfg, struct):
    NC = cfg.NC
    F1, F2, F3, WIN = cfg.F1, cfg.F2, cfg.F3, cfg.WIN
    ch_lo, ch_hi = struct["ch_lo"], struct["ch_hi"]
    LB, HB, MB = struct["LB"], struct["HB"], struct["MB"]
    pieces = struct["pieces"]
    totch = struct["totch"]
    tot_lo = struct["tot_lo"]
    HALVES = cfg.HALVES

    nc = bacc.Bacc("TRN2", target_bir_lowering=False, debug=False,
                   num_devices=NC)

    x_lo = nc.dram_tensor("x_lo", [cfg.LO, F1], F32R, kind="ExternalInput")
    x_hi = nc.dram_tensor("x_hi", [cfg.NPAD - cfg.LO, F1], F32R,
                          kind="ExternalInput")
    W1b = nc.dram_tensor("W1b", [F1, F2], BF16, kind="ExternalInput")
    W2b = nc.dram_tensor("W2b", [F2, F3], BF16, kind="ExternalInput")
    b1d = nc.dram_tensor("b1d", [F2, 1], F32, kind="ExternalInput")
    b2d = nc.dram_tensor("b2d", [F3, 1], F32, kind="ExternalInput")
    iota32 = nc.dram_tensor("iota32", [128, WIN], F32, kind="ExternalInput")
    iota16 = nc.dram_tensor("iota16", [128, WIN], BF16, kind="ExternalInput")
    ident16 = nc.dram_tensor("ident16", [128, 128], BF16, kind="ExternalInput")
    ident32 = nc.dram_tensor("ident32", [128, 128], F32, kind="ExternalInput")
    idx_d = nc.dram_tensor("idx", [128, totch * 8], I16, kind="ExternalInput")
    dstrow_d = nc.dram_tensor("dstrow", [128, totch], F32, kind="ExternalInput")
    wgt_d = nc.dram_tensor("wgt", [128, totch], F32, kind="ExternalInput")
    out_d = nc.dram_tensor("out", [128, HALVES, F3], F32, kind="ExternalOutput")

    with tile.TileContext(nc) as tc:
        with (
            tc.tile_pool(name="const", bufs=1) as cpool,
            tc.tile_pool(name="persist", bufs=1) as ppool,
            tc.tile_pool(name="dram", bufs=1, space="DRAM") as dpool,
        ):
            io32 = cpool.tile([128, WIN], F32)
            nc.sync.dma_start(out=io32[:], in_=iota32[:])
            io16 = cpool.tile([128, WIN], BF16)
            nc.sync.dma_start(out=io16[:], in_=iota16[:])
            w1 = cpool.tile([F1, F2], BF16)
            nc.sync.dma_start(out=w1[:], in_=W1b[:])
            w2 = cpool.tile([F2, F3], BF16)
            nc.sync.dma_start(out=w2[:], in_=W2b[:])
            b1 = cpool.tile([F2, 1], F32)
            nc.sync.dma_start(out=b1[:], in_=b1d[:])
            b2 = cpool.tile([F3, 1], F32)
            nc.sync.dma_start(out=b2[:], in_=b2d[:])
            id16 = cpool.tile([128, 128], BF16)
            nc.sync.dma_start(out=id16[:], in_=ident16[:])
            id32 = cpool.tile([128, 128], F32)
            nc.sync.dma_start(out=id32[:], in_=ident32[:])
            idxs = cpool.tile([128, totch * 8], I16)
            nc.sync.dma_start(out=idxs[:], in_=idx_d[:])
            dstrow = cpool.tile([128, totch], F32)
            nc.sync.dma_start(out=dstrow[:], in_=dstrow_d[:])
            wgt = cpool.tile([128, totch], F32)
            nc.sync.dma_start(out=wgt[:], in_=wgt_d[:])

            h_sb = ppool.tile([128, HALVES, F2], BF16)     # layer-1 output rows
            out_sb = ppool.tile([128, HALVES, F3], F32)    # layer-2 logits

            h_shard = dpool.tile([cfg.RPAD, F2], BF16)
            h_full = dpool.tile([cfg.NPAD, F2], BF16)

            def gather(dst_tile, in_ap, col0, n_chunks, elem):
                nc.gpsimd.dma_gather(
                    out_ap=dst_tile[:, :, :],
                    in_ap=in_ap,
                    idxs_ap=idxs[:, col0 * 8:(col0 + n_chunks) * 8],
                    num_idxs=n_chunks * 128,
                    num_idxs_reg=n_chunks * 128,
                    elem_size=elem,
                    single_packet=False,
                )

            def run_layer(layer):
                if layer == 1:
                    elem, gdt, src_lo, src_hi = F1, F32R, x_lo[:, :], x_hi[:, :]
                else:
                    elem, gdt = F2, BF16
                    src_lo = h_full[0:cfg.LO, :]
                    src_hi = h_full[cfg.LO:cfg.NPAD, :]
                with (
                    tc.tile_pool(name=f"g{layer}", bufs=3) as gpool,
                    tc.tile_pool(name=f"oh{layer}", bufs=8) as ohpool,
                    tc.tile_pool(name=f"ep{layer}", bufs=2) as eppool,
                    tc.tile_pool(name=f"ps{layer}", bufs=2, space="PSUM") as pspool,
                    tc.tile_pool(name=f"pt{layer}", bufs=2, space="PSUM") as ptpool,
                ):
                    for (w0, w1_) in pieces:
                        ncl = int(LB[w1_] - LB[w0])
                        nch = int(HB[w1_] - HB[w0])
                        g_lo = gpool.tile([128, ncl, elem], gdt, tag="glo")
                        g_hi = gpool.tile([128, nch, elem], gdt, tag="ghi")
                        gather(g_lo, src_lo, int(LB[w0]), ncl, elem)
                        gather(g_hi, src_hi, int(tot_lo + HB[w0]), nch, elem)
                        for w in range(w0, w1_):
                            nl, nh = int(ch_lo[w]), int(ch_hi[w])
                            if layer == 1:
                                acc = pspool.tile([F1, WIN], F32, tag="acc")
                            else:
                                acc = pspool.tile([F2, WIN], F32, tag="acc")
                            tot = nl + nh
                            for k in range(tot):
                                if k < nl:
                                    g = g_lo[:, int(LB[w] - LB[w0]) + k, :]
                                else:
                                    g = g_hi[:, int(HB[w] - HB[w0]) + k - nl, :]
                                mc = int(MB[w]) + k
                                if layer == 1:
                                    oh = ohpool.tile([128, WIN], F32R, tag="oh")
                                    nc.vector.tensor_scalar(
                                        out=oh[:], in0=io32[:],
                                        scalar1=dstrow[:, mc:mc + 1],
                                        scalar2=wgt[:, mc:mc + 1],
                                        op0=mybir.AluOpType.is_equal,
                                        op1=mybir.AluOpType.mult)
                                    nc.tensor.matmul(
                                        out=acc[:], lhsT=g, rhs=oh[:],
                                        start=(k == 0), stop=(k == tot - 1))
                                else:
                                    oh = ohpool.tile([128, WIN], BF16, tag="oh")
                                    nc.vector.tensor_scalar(
                                        out=oh[:], in0=io16[:],
                                        scalar1=dstrow[:, mc:mc + 1],
                                        scalar2=wgt[:, mc:mc + 1],
                                        op0=mybir.AluOpType.is_equal,
                                        op1=mybir.AluOpType.mult)
                                    nc.tensor.matmul(
                                        out=acc[:], lhsT=g, rhs=oh[:],
                                        start=(k == 0), stop=(k == tot - 1))
                            # window epilogue
                            import os as _os
                            if layer == 2 and _os.environ.get("K_STAGE") == "l2seg":
                                continue
                            if layer == 1:
                                mbf = eppool.tile([F1, WIN], BF16, tag="mbf")
                                nc.vector.tensor_copy(out=mbf[:], in_=acc[:])
                                z = ptpool.tile([F2, WIN], F32, tag="z")
                                nc.tensor.matmul(out=z[:], lhsT=w1[:],
                                                 rhs=mbf[:], start=True,
                                                 stop=True)
                                ht = eppool.tile([F2, WIN], BF16, tag="ht")
                                nc.scalar.activation(
                                    out=ht[:], in_=z[:],
                                    func=mybir.ActivationFunctionType.Relu,
                                    bias=b1[:, 0:1])
                                for hf in range(WIN // 128):
                                    hh = w * (WIN // 128) + hf
                                    if hh >= HALVES:
                                        continue
                                    tp = ptpool.tile([128, 128], BF16, tag="tp")
                                    nc.tensor.transpose(
                                        out=tp[:],
                                        in_=ht[:, hf * 128:(hf + 1) * 128],
                                        identity=id16[:])
                                    nc.vector.tensor_copy(
                                        out=h_sb[:, hh, :], in_=tp[:])
                            else:
                                mbf = eppool.tile([F2, WIN], BF16, tag="mbf")
                                nc.vector.tensor_copy(out=mbf[:], in_=acc[:])
                                z = ptpool.tile([F3, WIN], F32, tag="z")
                                nc.tensor.matmul(out=z[:], lhsT=w2[:],
                                                 rhs=mbf[:], start=True,
                                                 stop=True)
                                o2 = eppool.tile([128, WIN], F32, tag="ht")
                                nc.scalar.activation(
                                    out=o2[0:F3, :], in_=z[:],
                                    func=mybir.ActivationFunctionType.Identity,
                                    bias=b2[:, 0:1])
                                for hf in range(WIN // 128):
                                    hh = w * (WIN // 128) + hf
                                    if hh >= HALVES:
                                        continue
                                    tp = ptpool.tile([128, 128], F32, tag="tp")
                                    nc.tensor.transpose(
                                        out=tp[:],
                                        in_=o2[:, hf * 128:(hf + 1) * 128],
                                        identity=id32[:])
                                    nc.vector.tensor_copy(
                                        out=out_sb[:, hh, :], in_=tp[:, 0:F3])

            import os
            stage = os.environ.get("K_STAGE", "full")
            run_layer(1)
            if stage != "l1":
                # ship h to DRAM (strided: row hh*128+p), allgather, layer 2
                nc.sync.dma_start(
                    out=h_shard[:].rearrange("(hh p) f -> p hh f", p=128),
                    in_=h_sb[:])
                nc.gpsimd.collective_compute(
                    "AllGather",
                    mybir.AluOpType.bypass,
                    replica_groups=[list(range(NC))],
                    ins=[h_shard[:].opt()],
                    outs=[h_full[:].opt()],
                )
            if stage == "full":
                run_layer(2)

            # batched log_softmax over all halves: out = t - ln(sum(exp(t)))
            with tc.tile_pool(name="sm", bufs=1) as smp:
                mx = smp.tile([128, HALVES, 1], F32)
                nc.vector.reduce_max(out=mx[:], in_=out_sb[:],
                                     axis=mybir.AxisListType.X)
                nc.vector.tensor_tensor(
                    out=out_sb[:], in0=out_sb[:],
                    in1=mx[:].to_broadcast([128, HALVES, F3]),
                    op=mybir.AluOpType.subtract)
                ex = smp.tile([128, HALVES, F3], F32)
                nc.scalar.activation(out=ex[:], in_=out_sb[:],
                                     func=mybir.ActivationFunctionType.Exp)
                sm = smp.tile([128, HALVES, 1], F32)
                nc.vector.reduce_sum(out=sm[:], in_=ex[:],
                                     axis=mybir.AxisListType.X)
                ls = smp.tile([128, HALVES, 1], F32)
                nc.scalar.activation(out=ls[:], in_=sm[:],
                                     func=mybir.ActivationFunctionType.Ln)
                nc.vector.tensor_tensor(
                    out=out_sb[:], in0=out_sb[:],
                    in1=ls[:].to_broadcast([128, HALVES, F3]),
                    op=mybir.AluOpType.subtract)
                nc.sync.dma_start(out=out_d[:], in_=out_sb[:])

    nc.compile()
    return nc


_CACHE = {}


def _get_program(cfg, x, src, dst, W1, b1, W2, b2):
    deg = np.bincount(dst, minlength=cfg.N).astype(np.float64)
    deg_w = (1.0 / (deg + 1e-6)).astype(np.float32)
    struct, per_core = prep(cfg, src, dst, deg_w)

    xp = np.zeros((cfg.NPAD, cfg.F1), np.float32)
    for c in range(cfg.NC):
        xp[c * cfg.RPAD:c * cfg.RPAD + cfg.RPC] = x[c * cfg.RPC:(c + 1) * cfg.RPC]

    iota = np.arange(cfg.WIN, dtype=np.float32)
    shared = dict(
        x_lo=xp[:cfg.LO].copy(),
        x_hi=xp[cfg.LO:].copy(),
        W1b=W1.astype(ml_dtypes.bfloat16),
        W2b=W2.astype(ml_dtypes.bfloat16),
        b1d=b1.reshape(-1, 1).astype(np.float32),
        b2d=b2.reshape(-1, 1).astype(np.float32),
        iota32=np.tile(iota, (128, 1)),
        iota16=np.tile(iota, (128, 1)).astype(ml_dtypes.bfloat16),
        ident16=np.eye(128, dtype=ml_dtypes.bfloat16),
        ident32=np.eye(128, dtype=np.float32),
    )
    in_maps = []
    for c in range(cfg.NC):
        m = dict(shared)
        m["idx"] = per_core[c]["idx"]
        m["dstrow"] = per_core[c]["dstrow"]
        m["wgt"] = per_core[c]["wgt"]
        in_maps.append(m)

    key = (cfg.N, cfg.E, struct["totch"], tuple(struct["ch_lo"]),
           tuple(struct["ch_hi"]))
    if key not in _CACHE:
        _CACHE[key] = build_program(cfg, struct)
    return _CACHE[key], in_maps


def run(cfg, x, src, dst, W1, b1, W2, b2, trace=False, trace_kwargs=None):
    nc, in_maps = _get_program(cfg, x, src, dst, W1, b1, W2, b2)
    res = run_bass_kernel_spmd(nc, in_maps, core_ids=list(range(cfg.NC)),
                               trace=trace, **(trace_kwargs or {}))
    out = np.empty((cfg.N, cfg.F3), np.float32)
    for c in range(cfg.NC):
        o = np.asarray(res.results[c]["out"])  # [128, HALVES, F3]
        o = o.transpose(1, 0, 2).reshape(cfg.RPAD, cfg.F3)
        out[c * cfg.RPC:(c + 1) * cfg.RPC] = o[:cfg.RPC]
    return out, res


def kernel(x, src, dst, W1, b1, W2, b2):
    cfg = Cfg()
    out, _ = run(cfg, np.asarray(x, np.float32), np.asarray(src),
                 np.asarray(dst), np.asarray(W1, np.float32),
                 np.asarray(b1, np.float32), np.asarray(W2, np.float32),
                 np.asarray(b2, np.float32))
    return out

